# revision 36
# baseline (speedup 1.0000x reference)
"""CapsuleLayer (dynamic routing) Trainium2 kernel, v2.

Problem: B=128, I=1152 input capsules (A=8), O=10 output capsules (OA=16),
3 routing iterations.  Data-parallel over batch: 8 cores x 16 examples.

Per-core layout: SBUF partition p = is*16 + b (is = i mod 8, b = local batch),
half-chunk c = i // 8 in the free dim, vote coordinate n = oa*10 + o.

v2 structure (vs v1):
  - s1 matmuls interleaved with the votes matmuls in phase V (s1 needs no
    routing weights), so iteration 1 starts right after phase V.
  - delta (sum_oa votes*v) no longer uses a DVE tree-reduce: after the DVE
    broadcast-multiply tmp = votes*vrep, 16 accumulating identity-lhsT
    matmuls (one per oa, strided rhs) reduce over oa directly into a
    PSUM-resident logits tile.  logits += delta is free (PSUM accumulation
    across iterations, start=False).
  - softmax exp reads logits straight from PSUM; sqrt in squash is computed
    as exp(0.5*ln(x)) so Exp/Ln/Copy all live in one activation table set
    (zero ACT_TABLE_LOAD thrash).
  - everything pipelined in 3 c-blocks of 48 half-chunks.
"""

import numpy as np
import ml_dtypes

B, I, A, O, OA = 128, 1152, 8, 10, 16
NCORES = 8
BL = B // NCORES        # 16 examples per core
IS8 = 8                 # i-positions per half-chunk
C = I // IS8            # 144 half-chunks
CP = C // 2             # 72 paired chunks
N = O * OA              # 160, n = oa*O + o
N2 = 2 * N              # 320 per paired chunk
P = 128                 # p = is*BL + b
NUM_ROUTING = 3
CBN = 3                 # pipeline c-blocks
CBS = C // CBN          # 48 half-chunks per block
SW = 3                  # half-chunks per s-matmul
NS = C // SW            # 48 s-matmuls per iteration
NDMA = 12               # input DMA waves
GC = 0                  # half-chunks of each big multiply given to GPSIMD
DC = C - GC

_NC_CACHE = {}


def _build_nc():
    from contextlib import ExitStack

    import concourse.tile as tile
    import concourse.mybir as mybir
    from concourse import bacc

    F32 = mybir.dt.float32
    BF16 = mybir.dt.bfloat16
    F8 = mybir.dt.float8e4
    AF = mybir.ActivationFunctionType
    ALU = mybir.AluOpType
    AX = mybir.AxisListType

    nc = bacc.Bacc()
    xbd_d = nc.dram_tensor("xbd", [P, CP, P], BF16, kind="ExternalInput")
    xfl_d = nc.dram_tensor("xfl", [P, CP, BL], BF16, kind="ExternalInput")
    w2c_d = nc.dram_tensor("w2c", [P, CP, N2], BF16, kind="ExternalInput")
    bsel_d = nc.dram_tensor("bsel", [P, BL], BF16, kind="ExternalInput")
    brep_d = nc.dram_tensor("brep", [BL, P], BF16, kind="ExternalInput")
    bias_d = nc.dram_tensor("biasr", [BL, N], F32, kind="ExternalInput")
    id_d = nc.dram_tensor("id128", [P, P], BF16, kind="ExternalInput")
    vout_d = nc.dram_tensor("vout", [BL, N], F32, kind="ExternalOutput")

    with ExitStack() as ctx:
        tc = ctx.enter_context(tile.TileContext(nc))
        st = ctx.enter_context(tc.tile_pool(name="static", bufs=1))
        itp = ctx.enter_context(tc.tile_pool(name="itp", bufs=1))
        pps = ctx.enter_context(tc.tile_pool(name="pps", bufs=1, space="PSUM"))

        w2c = st.tile([P, CP, N2], BF16)
        votes = st.tile([P, C, N], BF16)
        big = st.tile([P, C, N], BF16)      # shared wv/tmp buffer
        expb = st.tile([P, C, O], BF16)
        route = st.tile([P, C, O], BF16)
        z = st.tile([P, C], F32)
        rz = st.tile([P, C], F32)
        bsel = st.tile([P, BL], BF16)
        brep = st.tile([BL, P], BF16)
        biasr = st.tile([BL, N], F32)
        id128 = st.tile([P, P], BF16)
        xfl = st.tile([P, CP, BL], BF16)

        # PSUM (bank = 512 f32): 3 logits banks + 1 s bank (persistent)
        lg0 = pps.tile([P, 512], F32, tag="lg0")
        lg1 = pps.tile([P, 512], F32, tag="lg1")
        lg2 = pps.tile([P, 512], F32, tag="lg2")
        lg = [lg0, lg1, lg2]
        s_ps = pps.tile([BL, 512], F32, tag="sps")

        dummy = st.tile([1, 1], F32)
        dz = st.tile([BL, 1], F32)
        dsq = st.tile([BL, 1], F32)

        v4 = votes[:].rearrange("p c (oa o) -> p c oa o", o=O)
        b4 = big[:].rearrange("p c (oa o) -> p c oa o", o=O)

        # ---- phase V: votes matmuls + interleaved s1 matmuls ----
        GRP = 2     # paired chunks per psum tile (1 bank each)
        with tc.tile_pool(name="ph1", bufs=1) as ph1, tc.tile_pool(
            name="psv", bufs=2, space="PSUM"
        ) as psv:
            xbd = ph1.tile([P, CP, P], BF16)
            nc.sync.dma_start(out=xfl[:], in_=xfl_d[:])
            nc.sync.dma_start(out=bsel[:], in_=bsel_d[:])
            nc.sync.dma_start(out=brep[:], in_=brep_d[:])
            nc.sync.dma_start(out=biasr[:], in_=bias_d[:])
            nc.sync.dma_start(out=id128[:], in_=id_d[:])
            nc.vector.memset(dz[:], 0.0)
            dstep = CP // NDMA
            for q in range(NDMA):
                sl = slice(q * dstep, (q + 1) * dstep)
                nc.sync.dma_start(out=xbd[:, sl, :], in_=xbd_d[:, sl, :])
                nc.sync.dma_start(out=w2c[:, sl, :], in_=w2c_d[:, sl, :])
            for g in range(CP // GRP):  # 36 groups of 2 pairs = 4 half-chunks
                ps = psv.tile([P, GRP * 512], F32, tag="pv")
                for j in range(GRP):
                    cp = g * GRP + j
                    nc.tensor.matmul(
                        ps[:, j * 512 : j * 512 + N2],
                        lhsT=xbd[:, cp, :],
                        rhs=w2c[:, cp, :],
                        start=True,
                        stop=True,
                    )
                    # s1 partial sums: same w2c rhs against a tiny stationary
                    # xfl (m=16, cheap LDWEIGHTS; route is uniform at t=1, the
                    # 1/O factor is folded into the s finalize).  out[b, h*N+n]
                    # accumulates the i-parity-h half of sum_i votes.
                    nc.tensor.matmul(
                        s_ps[:, 0:N2],
                        lhsT=xfl[:, cp, :],
                        rhs=w2c[:, cp, :],
                        start=(cp == 0),
                        stop=(cp == CP - 1),
                    )
                src = ps[:].rearrange("p (j s) -> p j s", j=GRP)[:, :, 0:N2]
                dst = votes[:, g * 2 * GRP : (g + 1) * 2 * GRP, :].rearrange(
                    "p (j c2) n -> p j (c2 n)", j=GRP
                )
                if g % 2 == 0:
                    nc.scalar.copy(dst, src)
                else:
                    nc.vector.tensor_copy(dst, src)
                if g == 20:
                    # prefetch the sqrt activation-table set (Copy stays valid
                    # in it) so squash1's Sqrt needs no critical-path reload
                    nc.scalar.activation(dsq[:], dz[:], AF.Sqrt)

        piv = ctx.enter_context(tc.tile_pool(name="piv", bufs=1, space="PSUM"))
        zps = piv.tile([P, 512], F32, tag="zps")

        # ---- routing iterations ----
        for t in range(1, NUM_ROUTING + 1):
            if t > 1:
                # softmax over o from PSUM-resident logits, per c-block
                for cb in range(CBN):
                    sl = slice(cb * CBS, (cb + 1) * CBS)
                    src = lg[cb][:, 0 : CBS * O].rearrange(
                        "p (c o) -> p c o", o=O
                    )
                    nc.scalar.activation(expb[:, sl], src, AF.Exp)
                # prefetch sqrt table for this iteration's squash: dsq (zeros
                # through Sqrt) is kept live by feeding it in as the real
                # sqrt's bias, so the table load hides under the wv phase
                nc.scalar.activation(dsq[:], dz[:], AF.Sqrt)
                # per c-block: z = sum_o expb via identity matmuls (PE),
                # then recip+route (DVE), then that block's wv pieces +
                # s matmuls -- no cross-block barrier anywhere
                r4 = route[:].unsqueeze(2).broadcast_to([P, C, OA, O])
                c0 = 0
                for cb in range(CBN):
                    sl = slice(cb * CBS, (cb + 1) * CBS)
                    zv = zps[:, cb * CBS : (cb + 1) * CBS]
                    for o in range(O):
                        nc.tensor.matmul(
                            zv,
                            lhsT=id128[:],
                            rhs=expb[:, sl, o],
                            start=(o == 0),
                            stop=(o == O - 1),
                        )
                    nc.vector.reciprocal_approx_fast(rz[:, sl], zv)
                    nc.vector.tensor_mul(
                        route[:, sl],
                        expb[:, sl],
                        rz[:, sl].unsqueeze(2).broadcast_to([P, CBS, O]),
                    )
                    for wps in (24, 24):
                        psl = slice(c0, c0 + wps)
                        nc.vector.tensor_mul(b4[:, psl], v4[:, psl], r4[:, psl])
                        for j in range(c0 // SW, (c0 + wps) // SW):
                            rhs = big[:, j * SW : (j + 1) * SW, :].rearrange(
                                "p c n -> p (c n)"
                            )
                            nc.tensor.matmul(
                                s_ps[:, 0 : SW * N],
                                lhsT=bsel[:],
                                rhs=rhs,
                                start=(j == 0),
                                stop=(j == NS - 1),
                            )
                        c0 += wps

            # ---- finalize s and squash ----
            sa = itp.tile([BL, N], F32, tag="sa")
            s_t = itp.tile([BL, N], F32, tag="stile")
            if t == 1:
                # s1 psum holds the two i-parity halves side by side
                nc.vector.tensor_copy(sa[:], s_ps[:, 0:N])
                nc.vector.tensor_add(sa[:], sa[:], s_ps[:, N:N2])
                nc.vector.scalar_tensor_tensor(
                    s_t[:], sa[:], 1.0 / O, biasr[:], op0=ALU.mult, op1=ALU.add
                )
            else:
                # sum the SW psum blocks with one strided reduce
                nc.vector.reduce_sum(
                    sa[:],
                    s_ps[:, 0 : SW * N].rearrange("b (c n) -> b n c", c=SW),
                    axis=AX.X,
                )
                nc.vector.tensor_add(s_t[:], sa[:], biasr[:])

            sq = itp.tile([BL, N], F32, tag="sq")
            nc.vector.tensor_mul(sq[:], s_t[:], s_t[:])
            nsq = itp.tile([BL, OA], F32, tag="nsq")
            nc.vector.reduce_sum(
                nsq[:], sq[:].rearrange("b (oa o) -> b oa o", o=O), axis=AX.X
            )
            nsq1 = itp.tile([BL, OA], F32, tag="nsq1")
            nc.vector.tensor_scalar_add(nsq1[:], nsq[:], 1.0)
            rn1 = itp.tile([BL, OA], F32, tag="rn1")
            nc.vector.reciprocal_approx_fast(rn1[:], nsq1[:])
            sr = itp.tile([BL, OA], F32, tag="sr")
            nc.scalar.activation(sr[:], nsq[:], AF.Sqrt, bias=dsq[:])
            f = itp.tile([BL, OA], F32, tag="f")
            nc.vector.tensor_mul(f[:], sr[:], rn1[:])
            if t == NUM_ROUTING:
                vt = itp.tile([BL, N], F32, tag="vt")
                nc.vector.tensor_mul(
                    vt[:].rearrange("b (oa o) -> b oa o", o=O),
                    s_t[:].rearrange("b (oa o) -> b oa o", o=O),
                    f[:].unsqueeze(2).broadcast_to([BL, OA, O]),
                )
                nc.sync.dma_start(out=vout_d[:], in_=vt[:])
                break

            # ---- replicate v to all partitions (vbf written directly) ----
            vbf = itp.tile([BL, N], BF16, tag="vbf")
            nc.vector.tensor_mul(
                vbf[:].rearrange("b (oa o) -> b oa o", o=O),
                s_t[:].rearrange("b (oa o) -> b oa o", o=O),
                f[:].unsqueeze(2).broadcast_to([BL, OA, O]),
            )
            vr_ps = piv.tile([P, 512], F32, tag="vrps")
            nc.tensor.matmul(
                vr_ps[:, 0:N], lhsT=brep[:], rhs=vbf[:], start=True, stop=True
            )
            vrep = itp.tile([P, N], BF16, tag="vrep")
            nc.scalar.copy(vrep[:], vr_ps[:, 0:N])

            # ---- tmp = votes*vrep (DVE) + oa-reduction into logits PSUM
            # via 16 accumulating identity matmuls per c-block (PE) ----
            vr_b = vrep[:].unsqueeze(1).broadcast_to([P, C, N])
            for cb in range(CBN):
                lo, hi = cb * CBS, (cb + 1) * CBS
                dhi = min(hi, DC)
                nc.vector.tensor_mul(
                    big[:, lo:dhi, :], votes[:, lo:dhi, :], vr_b[:, lo:dhi, :]
                )
                sl = slice(lo, hi)
                dst = lg[cb][:, 0 : CBS * O]
                for oa in range(OA):
                    nc.tensor.matmul(
                        dst,
                        lhsT=id128[:],
                        rhs=b4[:, sl, oa, :],
                        start=(t == 1 and oa == 0),
                        stop=(t == NUM_ROUTING - 1 and oa == OA - 1),
                        skip_group_check=True,
                    )

    nc.compile()
    return nc


def get_nc():
    if "nc" not in _NC_CACHE:
        _NC_CACHE["nc"] = _build_nc()
    return _NC_CACHE["nc"]


def make_in_maps(x, weights, biases):
    bf = ml_dtypes.bfloat16
    x = np.asarray(x, np.float32)
    weights = np.asarray(weights, np.float32)
    biases = np.asarray(biases, np.float32)

    # w2c[(h, is, a), cp, h2*N + (oa, o)] = w[(2cp+h)*8+is, a, o*16+oa] * (h==h2)
    w5 = (
        weights.reshape(CP, 2, IS8, A, O, OA)
        .transpose(0, 1, 2, 3, 5, 4)
        .reshape(CP, 2, IS8, A, N)
    )
    w2c = np.zeros((CP, 2, IS8, A, 2, N), np.float32)
    for h in range(2):
        w2c[:, h, :, :, h, :] = w5[:, h]
    w2c = w2c.reshape(CP, P, N2).transpose(1, 0, 2).astype(bf)

    eye = np.eye(BL, dtype=np.float32)
    bsel = np.tile(eye, (IS8, 1)).astype(bf)  # bsel[p, b'] = delta(p % BL == b')
    brep = np.tile(eye, (1, IS8)).astype(bf)  # brep[b, p] = delta(b == p % BL)
    biasr = np.broadcast_to(biases.T.reshape(1, N), (BL, N)).astype(np.float32).copy()
    id128 = np.eye(P, dtype=np.float32).astype(bf)

    in_maps = []
    idx = np.arange(IS8)
    for k in range(NCORES):
        xc = x[k * BL : (k + 1) * BL]  # [BL, I, A]
        xt = xc.reshape(BL, C, IS8, A).transpose(2, 1, 3, 0)  # [IS8, C, A, BL]
        xbd = np.zeros((C, IS8, A, IS8, BL), np.float32)
        xbd[:, idx, :, idx, :] = xt
        xbd = xbd.reshape(CP, 2 * IS8 * A, IS8 * BL).transpose(1, 0, 2).astype(bf)
        # xfl[(h, is, a), cp, b] = x[b, (2cp+h)*8+is, a]
        xfl = (
            xc.reshape(BL, CP, 2, IS8, A)
            .transpose(2, 3, 4, 1, 0)
            .reshape(P, CP, BL)
            .astype(bf)
        )
        in_maps.append(
            {
                "xbd": np.ascontiguousarray(xbd),
                "xfl": np.ascontiguousarray(xfl),
                "w2c": w2c,
                "bsel": bsel,
                "brep": brep,
                "biasr": biasr,
                "id128": id128,
            }
        )
    return in_maps


def assemble_out(results):
    out = np.zeros((B, 1, O, OA), np.float32)
    for k in range(NCORES):
        v = np.asarray(results[k]["vout"], np.float32)  # [BL, N], n = oa*O + o
        out[k * BL : (k + 1) * BL, 0] = v.reshape(BL, OA, O).transpose(0, 2, 1)
    return out


def kernel(x, weights, biases):
    from concourse.bass_utils import run_bass_kernel_spmd

    nc = get_nc()
    in_maps = make_in_maps(x, weights, biases)
    res = run_bass_kernel_spmd(nc, in_maps, list(range(NCORES)))
    return assemble_out(res.results)
